# revision 39
# baseline (speedup 1.0000x reference)
"""HGTConv Trainium2 kernel (8 NeuronCores, dst-sharded, batched SWDGE gather).

Math: softmax over the H=8 head axis followed by attn.mean(axis=-1) is
identically 1/8, so the attention branch drops out:

    out_dst = relu( segsum_dst(x_src[src]) @ Wbig * r8 + xres' )
    Wbig  = Wv @ Wm @ Wout
    r8    = 1/(8*max(cnt,1))                       (per dst node)
    xres' = x_dst + (cnt*r8)*bbig + bout           (host-folded residual)
    bbig  = (bv @ Wm + bm) @ Wout

Sharding: each core owns 1/8 of user dst nodes and 1/8 of game dst nodes,
and receives exactly the edges pointing into them. No collectives.

Gather: source rows are fetched with batched `dma_gather` (SWDGE custom
instruction, ~1us fixed cost per call amortized over thousands of rows)
instead of one indirect DMA per 128 rows. int16 gather indices limit the
addressable range to 32768 rows, so each source table is split into banks
of 25000 rows and one gather per (segment, bank) is issued.

Dst-node-to-tile assignment is bin-packed on the host (LPT) so that each
(tile, bank) group holds at most 128 edges (one 128-slot chunk); a few
"fat" tiles with a 256-edge budget absorb heavy nodes. This gives a fixed
chunk structure shared by all 8 cores (single SPMD program) at ~90-95%
gather-slot utilization.

Scatter within a tile: one-hot matmul. For each chunk, a [128 slots, 128
dst] selection matrix M (DVE is_equal of per-slot local-dst vs an iota
row) turns the segment-sum into PE matmuls accumulating S^T in PSUM; the
r8 scale rides the PSUM->SBUF copy (free-dim multiply with a
host-replicated r8 row); then a fused (Wbig) matmul, residual add, relu.
"""

import math
import os
from contextlib import ExitStack

import numpy as np
import ml_dtypes

import concourse.bass as bass
import concourse.tile as tile
import concourse.mybir as mybir
from concourse import bacc
from concourse.bass_utils import run_bass_kernel_spmd

P = 128
D = 256
BF16 = ml_dtypes.bfloat16

# full-size problem config; side u: dst=user src=game, side g: dst=game src=user
CFG_FULL = dict(
    ncores=8,
    n_user=100000,
    n_game=50000,
    u=dict(T=99, fat=4, nbanks=2, nsegs=6),
    g=dict(T=50, fat=4, nbanks=4, nsegs=6),
)


class PackError(Exception):
    pass


_DEBUG_BUILD = False  # set True for CoreSim runs (keeps debug info)


# ------------------------------------------------------------ structure

def _side_structure(scfg):
    """Core-independent chunk/event layout for one side."""
    T, fat, B, nsegs = scfg["T"], scfg["fat"], scfg["nbanks"], scfg["nsegs"]
    nch_tile = [2 if t >= T - fat else 1 for t in range(T)]  # chunks per (t, b)

    # segments: contiguous tile ranges, balanced
    bounds = [round(i * T / nsegs) for i in range(nsegs + 1)]
    segs = [(bounds[i], bounds[i + 1]) for i in range(nsegs)]

    # event order (ld columns / matmul order): tile-major
    ev_of = {}
    n_ev = 0
    for t in range(T):
        for b in range(B):
            for j in range(nch_tile[t]):
                ev_of[(t, b, j)] = n_ev
                n_ev += 1

    # gather-position order: seg-major, bank-major within seg
    gpos_of = {}
    seg_info = []  # per seg: (tile_lo, tile_hi, gpos_lo, nch_seg, [(b, gpos_lo_b, nch_b)])
    pos = 0
    for lo, hi in segs:
        gl = pos
        per_bank = []
        for b in range(B):
            bl = pos
            for t in range(lo, hi):
                for j in range(nch_tile[t]):
                    gpos_of[(t, b, j)] = pos
                    pos += 1
            per_bank.append((b, bl, pos - bl))
        seg_info.append((lo, hi, gl, pos - gl, per_bank))
    n_pos = pos

    return dict(
        T=T, fat=fat, B=B, nch_tile=nch_tile, segs=segs,
        ev_of=ev_of, n_ev=n_ev, gpos_of=gpos_of, n_pos=n_pos,
        seg_info=seg_info,
    )


def _structures(cfg):
    return dict(u=_side_structure(cfg["u"]), g=_side_structure(cfg["g"]))


# ------------------------------------------------------------ host packing

def _lpt_binpack(cnt, T, nch_tile, cap_chunk=128):
    """Assign each dst node to a tile s.t. per-(tile,bank) load <= cap and
    <=128 nodes per tile. cnt: [n_nodes, B] int. Returns tile_of [n_nodes]."""
    n, B = cnt.shape
    if n > T * P:
        raise PackError(f"{n} nodes > {T * P} slots")
    caps = np.broadcast_to(
        (np.array(nch_tile, np.int64) * cap_chunk)[:, None], (T, B)
    ).copy()
    slots = np.full(T, P, np.int64)
    tile_of = np.full(n, -1, np.int64)
    tot = cnt.sum(1)
    order = np.argsort(-cnt.max(1), kind="stable")
    nz = order[tot[order] > 0]
    for node in nz:
        c = cnt[node]
        feas = (slots > 0) & (caps >= c).all(1)
        if not feas.any():
            raise PackError("no feasible tile (escalate fat budget)")
        score = caps.sum(1).astype(np.float64)
        score[~feas] = -1.0
        t = int(np.argmax(score))
        tile_of[node] = t
        caps[t] -= c
        slots[t] -= 1
    z = order[tot[order] == 0]
    zi = 0
    for t in range(T):
        k = int(slots[t])
        if k > 0 and zi < len(z):
            take = z[zi : zi + k]
            tile_of[take] = t
            slots[t] -= len(take)
            zi += len(take)
    if zi < len(z):
        raise PackError("not enough node slots")
    return tile_of


def _pack_side(st, dst_local, src, bank_sz, n_dst_slice, x_dst, Wbig, bbig, bout):
    """Host packing of one core-side. Returns dict of device arrays + unpack maps."""
    T, B, n_ev, n_pos = st["T"], st["B"], st["n_ev"], st["n_pos"]
    nch_tile, ev_of, gpos_of = st["nch_tile"], st["ev_of"], st["gpos_of"]

    bank = (src // bank_sz).astype(np.int64)
    sib = (src - bank * bank_sz).astype(np.int64)  # src row within bank

    cnt = np.zeros((n_dst_slice, B), np.int64)
    np.add.at(cnt, (dst_local, bank), 1)
    tile_of = _lpt_binpack(cnt, T, st["nch_tile"])

    # slot within tile: stable order by node id
    order = np.argsort(tile_of, kind="stable")
    slot_of = np.empty(n_dst_slice, np.int64)
    tile_sorted = tile_of[order]
    first = np.searchsorted(tile_sorted, np.arange(T))
    slot_of[order] = np.arange(n_dst_slice) - first[tile_sorted]
    assert slot_of.max() < P

    # group edges by (tile, bank), sorted by src row for DMA locality
    g = tile_of[dst_local] * B + bank
    eorder = np.lexsort((sib, g))
    gs = g[eorder]
    ds = dst_local[eorder]
    ss = sib[eorder]
    gfirst = np.searchsorted(gs, np.arange(T * B))
    gcount = np.diff(np.searchsorted(gs, np.arange(T * B + 1)))
    within = np.arange(len(gs)) - gfirst[gs]

    t_e = gs // B
    b_e = gs % B
    j_e = within >> 7
    p_e = within & 127
    nch_e = np.array(st["nch_tile"], np.int64)[t_e]
    if (j_e >= nch_e).any():
        raise PackError("chunk budget overflow")

    gpos_tab = np.zeros((T, B, 2), np.int64)
    ev_tab = np.zeros((T, B, 2), np.int64)
    for (t, b, j), v in gpos_of.items():
        gpos_tab[t, b, j] = v
    for (t, b, j), v in ev_of.items():
        ev_tab[t, b, j] = v
    gpos_e = gpos_tab[t_e, b_e, j_e]
    ev_e = ev_tab[t_e, b_e, j_e]

    # gather indices (global gather position i -> src row in bank), pad = 0
    idx_flat = np.zeros(n_pos * P, np.int64)
    idx_flat[gpos_e * P + p_e] = ss
    # int16 layout [128, n_pos*8]: value i at [i%16, i//16], replicated x8
    idx16 = np.zeros((P, n_pos * 8), np.int16)
    block = idx_flat.reshape(-1, 16).T.astype(np.int16)  # [16, n_pos*8]
    for k in range(8):
        idx16[k * 16 : (k + 1) * 16] = block

    # ld: local dst slot per chunk slot, -1 for dummies
    ld = np.full((P, n_ev), -1.0, np.float32)
    ld[p_e, ev_e] = slot_of[ds].astype(np.float32)

    # per-node scales
    ctot = cnt.sum(1).astype(np.float32)
    r8 = 1.0 / (8.0 * np.maximum(ctot, 1.0))

    # r8 replicated row per (tile, slot); holes -> 1/8 (slot output discarded)
    r8row = np.full(T * P, 1.0 / 8.0, np.float32)
    r8row[tile_of * P + slot_of] = r8
    r8rep = np.broadcast_to(r8row[None, :], (P, T * P))

    # folded residual in pm layout [slot, tile*D]
    xres_n = x_dst + (ctot * r8)[:, None] * bbig[None, :] + bout[None, :]
    xres = np.zeros((P, T * D), np.float32)
    xres[slot_of[:, None], (tile_of * D)[:, None] + np.arange(D)[None, :]] = xres_n

    return dict(
        idx=idx16,
        ld=np.ascontiguousarray(ld.astype(BF16)),
        r8rep=np.ascontiguousarray(r8rep.astype(BF16)),
        xres=np.ascontiguousarray(xres.astype(BF16)),
        tile_of=tile_of,
        slot_of=slot_of,
    )


def _fold_weights(Wv, bv, Wm, bm, Wout, bout):
    Wbig = (np.float32(Wv) @ np.float32(Wm)) @ np.float32(Wout)
    bbig = (np.float32(bv) @ np.float32(Wm) + np.float32(bm)) @ np.float32(Wout)
    return np.ascontiguousarray(Wbig).astype(BF16), bbig, np.float32(bout)


# ------------------------------------------------------------ device build

def _build(cfg, sts):
    f32 = mybir.dt.float32
    bf = mybir.dt.bfloat16
    i16 = mybir.dt.int16

    nc = bacc.Bacc(
        "TRN2",
        target_bir_lowering=False,
        debug=_DEBUG_BUILD,
        num_devices=cfg["ncores"],
        num_swdge_queues=4,
    )

    bank_cfg = dict(
        u=("xg", cfg["n_game"]),   # side u gathers from game banks
        g=("xu", cfg["n_user"]),   # side g gathers from user banks
    )
    sides = []
    for name in ("u", "g"):
        st = sts[name]
        pre, n_src = bank_cfg[name]
        B = st["B"]
        bank_sz = n_src // B
        side = dict(name=name, st=st, bank_sz=bank_sz)
        side["banks"] = [
            nc.dram_tensor(f"{pre}{b}", [bank_sz, D], bf, kind="ExternalInput")
            for b in range(B)
        ]
        T = st["T"]
        side["idx"] = nc.dram_tensor(f"idx_{name}", [P, st["n_pos"] * 8], i16, kind="ExternalInput")
        side["ld"] = nc.dram_tensor(f"ld_{name}", [P, st["n_ev"]], bf, kind="ExternalInput")
        side["r8"] = nc.dram_tensor(f"r8_{name}", [P, T * P], bf, kind="ExternalInput")
        side["xres"] = nc.dram_tensor(f"xres_{name}", [P, T * D], bf, kind="ExternalInput")
        side["w"] = nc.dram_tensor(f"w_{name}", [D, D], bf, kind="ExternalInput")
        side["out"] = nc.dram_tensor(f"out_{name}", [P, T * D], bf, kind="ExternalOutput")
        sides.append(side)

    iota_hbm = nc.dram_tensor("iota", [P, P], bf, kind="ExternalInput")
    ident_hbm = nc.dram_tensor("ident", [P, P], bf, kind="ExternalInput")

    max_seg_ch = max(
        info[3] for side in sides for info in side["st"]["seg_info"]
    )

    with tile.TileContext(nc) as tc, ExitStack() as ctx:
        const = ctx.enter_context(tc.tile_pool(name="const", bufs=1))
        gpool = ctx.enter_context(tc.tile_pool(name="gpool", bufs=16))
        mp = ctx.enter_context(tc.tile_pool(name="mp", bufs=2))
        stbp = ctx.enter_context(tc.tile_pool(name="stbp", bufs=3))
        xrp = ctx.enter_context(tc.tile_pool(name="xrp", bufs=2))
        oop = ctx.enter_context(tc.tile_pool(name="oop", bufs=3))
        outp = ctx.enter_context(tc.tile_pool(name="outp", bufs=2))
        st_ps = ctx.enter_context(tc.tile_pool(name="st_ps", bufs=4, space="PSUM"))
        op_ps = ctx.enter_context(tc.tile_pool(name="op_ps", bufs=3, space="PSUM"))

        iota_res = const.tile([P, P], bf, tag="iota", name="iota_res")
        nc.sync.dma_start(iota_res[:], iota_hbm[:])
        ident_res = const.tile([P, P], bf, tag="ident", name="ident_res")
        nc.sync.dma_start(ident_res[:], ident_hbm[:])
        gq = [0]  # round-robin SWDGE queue assignment for gathers

        for side in sides:
            st, n = side["st"], side["name"]
            side["idx_res"] = const.tile([P, st["n_pos"] * 8], i16, tag=f"idx_{n}", name=f"idx_res_{n}")
            nc.sync.dma_start(side["idx_res"][:], side["idx"][:])
            side["ld_res"] = const.tile([P, st["n_ev"]], bf, tag=f"ld_{n}", name=f"ld_res_{n}")
            nc.sync.dma_start(side["ld_res"][:], side["ld"][:])
            side["r8_res"] = const.tile([P, st["T"] * P], bf, tag=f"r8_{n}", name=f"r8_res_{n}")
            nc.sync.dma_start(side["r8_res"][:], side["r8"][:])
            side["w0"] = const.tile([P, D], bf, tag=f"w0_{n}", name=f"w0_{n}")
            nc.sync.dma_start(side["w0"][:], side["w"][0:P, :])
            side["w1"] = const.tile([P, D], bf, tag=f"w1_{n}", name=f"w1_{n}")
            nc.sync.dma_start(side["w1"][:], side["w"][P:D, :])

        for side in sides:
            st, n = side["st"], side["name"]
            T, B = st["T"], st["B"]
            nch_tile, ev_of, gpos_of = st["nch_tile"], st["ev_of"], st["gpos_of"]
            idx_res, ld_res, r8_res = side["idx_res"], side["ld_res"], side["r8_res"]

            MAXCH = 8  # dma_gather HW ucode limit: 1024 idxs per instruction
            for (tlo, thi, gl, nch_seg, per_bank) in st["seg_info"]:
                ntile = thi - tlo
                # one pool tile per gather so writers are independent and the
                # 4 SWDGE queues' descriptor generation actually overlaps
                gtiles = {}
                bl_of = {}
                for (b, bl, nch_b) in per_bank:
                    bl_of[b] = bl
                    for si, off in enumerate(range(0, nch_b, MAXCH)):
                        nch = min(MAXCH, nch_b - off)
                        lo = bl + off
                        gt = gpool.tile([P, MAXCH * D], bf, tag="gbuf", name=f"gbuf_{n}")
                        gtiles[(b, si)] = gt
                        out3 = gt[:, 0 : nch * D].rearrange("p (c e) -> p c e", e=D)
                        nc.gpsimd.dma_gather(
                            out3,
                            side["banks"][b][:, :],
                            idx_res[:, lo * 8 : (lo + nch) * 8],
                            nch * P,
                            nch * P,
                            D,
                            queue_num=gq[0] % 4,
                        )
                        gq[0] += 1

                xr = xrp.tile([P, ntile * D], bf, tag="xr", name=f"xr_{n}")
                nc.sync.dma_start(xr[:], side["xres"][:, tlo * D : thi * D])
                og = outp.tile([P, ntile * D], bf, tag="og", name=f"og_{n}")

                # one-hot selection matrices for the whole segment in one DVE op
                ev0s = ev_of[(tlo, 0, 0)]
                ev1s = ev_of[(thi - 1, B - 1, nch_tile[thi - 1] - 1)] + 1
                nevs = ev1s - ev0s
                Mt = mp.tile([P, nevs * P], bf, tag="m", name=f"m_{n}")
                nc.vector.tensor_tensor(
                    out=Mt[:].rearrange("p (c j) -> p c j", j=P),
                    in0=ld_res[:, ev0s:ev1s].unsqueeze(2).to_broadcast([P, nevs, P]),
                    in1=iota_res[:].unsqueeze(1).to_broadcast([P, nevs, P]),
                    op=mybir.AluOpType.is_equal,
                )

                def do_tile(t, opre_ap):
                    """stage-1 scatter matmuls + r8 scale + stage-2 + residual"""
                    Ct = nch_tile[t] * B
                    ti_ = t - tlo
                    stp = st_ps.tile([P, D], f32, tag="st")
                    for h in range(2):  # feature halves: sequential PSUM groups
                        k = 0
                        for b in range(B):
                            for j in range(nch_tile[t]):
                                lp = gpos_of[(t, b, j)] - bl_of[b]
                                kM = ev_of[(t, b, j)] - ev0s
                                gt = gtiles[(b, lp // MAXCH)]
                                lc = lp % MAXCH
                                X = gt[:, lc * D + h * P : lc * D + (h + 1) * P]
                                nc.tensor.matmul(
                                    stp[:, h * P : (h + 1) * P], lhsT=X,
                                    rhs=Mt[:, kM * P : (kM + 1) * P],
                                    start=(k == 0), stop=(k == Ct - 1),
                                )
                                k += 1
                    stb = stbp.tile([P, D], bf, tag="stb", name=f"stb_{n}")
                    nc.vector.tensor_tensor(
                        out=stb[:].rearrange("p (h j) -> p h j", j=P),
                        in0=stp[:].rearrange("p (h j) -> p h j", j=P),
                        in1=r8_res[:, t * P : (t + 1) * P].unsqueeze(1).to_broadcast([P, 2, P]),
                        op=mybir.AluOpType.mult,
                    )
                    nc.tensor.matmul(opre_ap, lhsT=stb[:, 0:P], rhs=side["w0"][:], start=True, stop=False)
                    nc.tensor.matmul(opre_ap, lhsT=stb[:, P:D], rhs=side["w1"][:], start=False, stop=True)

                for t0 in range(tlo, thi, 2):
                    npair = min(2, thi - t0)
                    W = npair * D
                    ti = t0 - tlo
                    opre = op_ps.tile([P, W], f32, tag="opre")
                    for q in range(npair):
                        do_tile(t0 + q, opre[:, q * D : (q + 1) * D])
                    oo = oop.tile([P, W], f32, tag="oo", name=f"oo_{n}")
                    nc.vector.tensor_tensor(
                        out=oo[:], in0=opre[:], in1=xr[:, ti * D : ti * D + W],
                        op=mybir.AluOpType.add,
                    )
                    nc.scalar.activation(
                        og[:, ti * D : ti * D + W], oo[:],
                        mybir.ActivationFunctionType.Relu,
                    )

                nc.sync.dma_start(side["out"][:, tlo * D : thi * D], og[:])

    nc.compile()
    return nc


_NC_CACHE = {}


def _cfg_key(cfg):
    return (
        cfg["ncores"], cfg["n_user"], cfg["n_game"],
        tuple(sorted(cfg["u"].items())), tuple(sorted(cfg["g"].items())),
    )


def _get_nc(cfg, sts):
    key = _cfg_key(cfg)
    if key not in _NC_CACHE:
        _NC_CACHE[key] = _build(cfg, sts)
    return _NC_CACHE[key]


# ------------------------------------------------------------------- driver

def _prepare(inputs, cfg):
    ncores = cfg["ncores"]
    n_user, n_game = cfg["n_user"], cfg["n_game"]
    uslice, gslice = n_user // ncores, n_game // ncores

    Wb_u, bb_u, bo_u = _fold_weights(
        inputs["Wv_game"], inputs["bv_game"], inputs["Wm_rev"], inputs["bm_rev"],
        inputs["Wout_user"], inputs["bout_user"],
    )
    Wb_g, bb_g, bo_g = _fold_weights(
        inputs["Wv_user"], inputs["bv_user"], inputs["Wm_played"], inputs["bm_played"],
        inputs["Wout_game"], inputs["bout_game"],
    )

    x_user = np.ascontiguousarray(np.float32(inputs["x_user"]))
    x_game = np.ascontiguousarray(np.float32(inputs["x_game"]))
    xu_bf = x_user.astype(BF16)
    xg_bf = x_game.astype(BF16)

    ep_s = np.asarray(inputs["ei_played_src"]).astype(np.int64)
    ep_d = np.asarray(inputs["ei_played_dst"]).astype(np.int64)
    er_s = np.asarray(inputs["ei_rev_src"]).astype(np.int64)
    er_d = np.asarray(inputs["ei_rev_dst"]).astype(np.int64)

    while True:
        sts = _structures(cfg)
        try:
            in_maps = []
            packs = []
            for k in range(ncores):
                sel_u = (er_d >= k * uslice) & (er_d < (k + 1) * uslice)
                pu = _pack_side(
                    sts["u"], er_d[sel_u] - k * uslice, er_s[sel_u],
                    n_game // cfg["u"]["nbanks"], uslice,
                    x_user[k * uslice : (k + 1) * uslice],
                    np.float32(Wb_u), bb_u, bo_u,
                )
                sel_g = (ep_d >= k * gslice) & (ep_d < (k + 1) * gslice)
                pg = _pack_side(
                    sts["g"], ep_d[sel_g] - k * gslice, ep_s[sel_g],
                    n_user // cfg["g"]["nbanks"], gslice,
                    x_game[k * gslice : (k + 1) * gslice],
                    np.float32(Wb_g), bb_g, bo_g,
                )
                packs.append((pu, pg))
                im = dict(
                    iota=np.ascontiguousarray(
                        np.broadcast_to(np.arange(P, dtype=np.float32)[None, :], (P, P))
                    ).astype(BF16),
                    ident=np.eye(P, dtype=np.float32).astype(BF16),
                    idx_u=pu["idx"], ld_u=pu["ld"], r8_u=pu["r8rep"], xres_u=pu["xres"],
                    w_u=Wb_u,
                    idx_g=pg["idx"], ld_g=pg["ld"], r8_g=pg["r8rep"], xres_g=pg["xres"],
                    w_g=Wb_g,
                )
                ubank = n_game // cfg["u"]["nbanks"]
                for b in range(cfg["u"]["nbanks"]):
                    im[f"xg{b}"] = xg_bf[b * ubank : (b + 1) * ubank]
                gbank = n_user // cfg["g"]["nbanks"]
                for b in range(cfg["g"]["nbanks"]):
                    im[f"xu{b}"] = xu_bf[b * gbank : (b + 1) * gbank]
                in_maps.append(im)
            break
        except PackError:
            # escalate fat-tile budget (changes structure => recompile)
            cfg = dict(cfg, u=dict(cfg["u"]), g=dict(cfg["g"]))
            cfg["u"]["fat"] += 2
            cfg["u"]["T"] += 1
            cfg["g"]["fat"] += 2
            cfg["g"]["T"] += 1

    return cfg, sts, in_maps, packs


def _run(inputs, cfg=None, trace=False, **run_kwargs):
    cfg = cfg or CFG_FULL
    cfg, sts, in_maps, packs = _prepare(inputs, cfg)
    ncores = cfg["ncores"]
    uslice, gslice = cfg["n_user"] // ncores, cfg["n_game"] // ncores

    nc = _get_nc(cfg, sts)
    res = run_bass_kernel_spmd(nc, in_maps, list(range(ncores)), trace=trace, **run_kwargs)

    def unpack(a, pack, T, nrows):
        a3 = np.asarray(a, dtype=np.float32).reshape(P, T, D)
        return a3[pack["slot_of"], pack["tile_of"], :]

    out_user = np.concatenate(
        [unpack(res.results[k]["out_u"], packs[k][0], sts["u"]["T"], uslice) for k in range(ncores)],
        axis=0,
    )
    out_game = np.concatenate(
        [unpack(res.results[k]["out_g"], packs[k][1], sts["g"]["T"], gslice) for k in range(ncores)],
        axis=0,
    )
    full = np.concatenate([out_user, out_game], axis=0).astype(np.float32)
    return full, res


def kernel(**inputs) -> np.ndarray:
    out, _ = _run(inputs)
    return out


# revision 41
# speedup vs baseline: 1.0042x; 1.0042x over previous
"""HGTConv Trainium2 kernel (8 NeuronCores, dst-sharded, batched SWDGE gather).

Math: softmax over the H=8 head axis followed by attn.mean(axis=-1) is
identically 1/8, so the attention branch drops out:

    out_dst = relu( segsum_dst(x_src[src]) @ Wbig * r8 + xres' )
    Wbig  = Wv @ Wm @ Wout
    r8    = 1/(8*max(cnt,1))                       (per dst node)
    xres' = x_dst + (cnt*r8)*bbig + bout           (host-folded residual)
    bbig  = (bv @ Wm + bm) @ Wout

Sharding: each core owns 1/8 of user dst nodes and 1/8 of game dst nodes,
and receives exactly the edges pointing into them. No collectives.

Gather: source rows are fetched with batched `dma_gather` (SWDGE custom
instruction, ~1us fixed cost per call amortized over thousands of rows)
instead of one indirect DMA per 128 rows. int16 gather indices limit the
addressable range to 32768 rows, so each source table is split into banks
of 25000 rows and one gather per (segment, bank) is issued.

Dst-node-to-tile assignment is bin-packed on the host (LPT) so that each
(tile, bank) group holds at most 128 edges (one 128-slot chunk); a few
"fat" tiles with a 256-edge budget absorb heavy nodes. This gives a fixed
chunk structure shared by all 8 cores (single SPMD program) at ~90-95%
gather-slot utilization.

Scatter within a tile: one-hot matmul. For each chunk, a [128 slots, 128
dst] selection matrix M (DVE is_equal of per-slot local-dst vs an iota
row) turns the segment-sum into PE matmuls accumulating S^T in PSUM; the
r8 scale rides the PSUM->SBUF copy (free-dim multiply with a
host-replicated r8 row); then a fused (Wbig) matmul, residual add, relu.
"""

import math
import os
from contextlib import ExitStack

import numpy as np
import ml_dtypes

import concourse.bass as bass
import concourse.tile as tile
import concourse.mybir as mybir
from concourse import bacc
from concourse.bass_utils import run_bass_kernel_spmd

P = 128
D = 256
BF16 = ml_dtypes.bfloat16

# full-size problem config; side u: dst=user src=game, side g: dst=game src=user
CFG_FULL = dict(
    ncores=8,
    n_user=100000,
    n_game=50000,
    u=dict(T=99, fat=4, nbanks=2, nsegs=6),
    g=dict(T=50, fat=4, nbanks=4, nsegs=6),
)


class PackError(Exception):
    pass


_DEBUG_BUILD = False  # set True for CoreSim runs (keeps debug info)


# ------------------------------------------------------------ structure

def _side_structure(scfg):
    """Core-independent chunk/event layout for one side."""
    T, fat, B, nsegs = scfg["T"], scfg["fat"], scfg["nbanks"], scfg["nsegs"]
    # fat tiles at the FRONT so the tail segments are light
    nch_tile = [2 if t < fat else 1 for t in range(T)]  # chunks per (t, b)

    # segments: contiguous tile ranges, tail segments small to shorten the
    # compute drain after the last gather completes
    w = [1.0] * (nsegs - 2) + [0.5, 0.25]
    cw = np.cumsum([0.0] + w) / sum(w)
    bounds = sorted({int(round(c * T)) for c in cw} | {0, T})
    segs = [(bounds[i], bounds[i + 1]) for i in range(len(bounds) - 1)]

    # event order (ld columns / matmul order): tile-major
    ev_of = {}
    n_ev = 0
    for t in range(T):
        for b in range(B):
            for j in range(nch_tile[t]):
                ev_of[(t, b, j)] = n_ev
                n_ev += 1

    # gather-position order: seg-major, bank-major within seg
    gpos_of = {}
    seg_info = []  # per seg: (tile_lo, tile_hi, gpos_lo, nch_seg, [(b, gpos_lo_b, nch_b)])
    pos = 0
    for lo, hi in segs:
        gl = pos
        per_bank = []
        for b in range(B):
            bl = pos
            for t in range(lo, hi):
                for j in range(nch_tile[t]):
                    gpos_of[(t, b, j)] = pos
                    pos += 1
            per_bank.append((b, bl, pos - bl))
        seg_info.append((lo, hi, gl, pos - gl, per_bank))
    n_pos = pos

    return dict(
        T=T, fat=fat, B=B, nch_tile=nch_tile, segs=segs,
        ev_of=ev_of, n_ev=n_ev, gpos_of=gpos_of, n_pos=n_pos,
        seg_info=seg_info,
    )


def _structures(cfg):
    return dict(u=_side_structure(cfg["u"]), g=_side_structure(cfg["g"]))


# ------------------------------------------------------------ host packing

def _lpt_binpack(cnt, T, nch_tile, cap_chunk=128):
    """Assign each dst node to a tile s.t. per-(tile,bank) load <= cap and
    <=128 nodes per tile. cnt: [n_nodes, B] int. Returns tile_of [n_nodes]."""
    n, B = cnt.shape
    if n > T * P:
        raise PackError(f"{n} nodes > {T * P} slots")
    caps = np.broadcast_to(
        (np.array(nch_tile, np.int64) * cap_chunk)[:, None], (T, B)
    ).copy()
    slots = np.full(T, P, np.int64)
    tile_of = np.full(n, -1, np.int64)
    tot = cnt.sum(1)
    order = np.argsort(-cnt.max(1), kind="stable")
    nz = order[tot[order] > 0]
    for node in nz:
        c = cnt[node]
        feas = (slots > 0) & (caps >= c).all(1)
        if not feas.any():
            raise PackError("no feasible tile (escalate fat budget)")
        score = caps.sum(1).astype(np.float64)
        score[~feas] = -1.0
        t = int(np.argmax(score))
        tile_of[node] = t
        caps[t] -= c
        slots[t] -= 1
    z = order[tot[order] == 0]
    zi = 0
    for t in range(T):
        k = int(slots[t])
        if k > 0 and zi < len(z):
            take = z[zi : zi + k]
            tile_of[take] = t
            slots[t] -= len(take)
            zi += len(take)
    if zi < len(z):
        raise PackError("not enough node slots")
    return tile_of


def _pack_side(st, dst_local, src, bank_sz, n_dst_slice, x_dst, Wbig, bbig, bout):
    """Host packing of one core-side. Returns dict of device arrays + unpack maps."""
    T, B, n_ev, n_pos = st["T"], st["B"], st["n_ev"], st["n_pos"]
    nch_tile, ev_of, gpos_of = st["nch_tile"], st["ev_of"], st["gpos_of"]

    bank = (src // bank_sz).astype(np.int64)
    sib = (src - bank * bank_sz).astype(np.int64)  # src row within bank

    cnt = np.zeros((n_dst_slice, B), np.int64)
    np.add.at(cnt, (dst_local, bank), 1)
    tile_of = _lpt_binpack(cnt, T, st["nch_tile"])

    # slot within tile: stable order by node id
    order = np.argsort(tile_of, kind="stable")
    slot_of = np.empty(n_dst_slice, np.int64)
    tile_sorted = tile_of[order]
    first = np.searchsorted(tile_sorted, np.arange(T))
    slot_of[order] = np.arange(n_dst_slice) - first[tile_sorted]
    assert slot_of.max() < P

    # group edges by (tile, bank), sorted by src row for DMA locality
    g = tile_of[dst_local] * B + bank
    eorder = np.lexsort((sib, g))
    gs = g[eorder]
    ds = dst_local[eorder]
    ss = sib[eorder]
    gfirst = np.searchsorted(gs, np.arange(T * B))
    gcount = np.diff(np.searchsorted(gs, np.arange(T * B + 1)))
    within = np.arange(len(gs)) - gfirst[gs]

    t_e = gs // B
    b_e = gs % B
    j_e = within >> 7
    p_e = within & 127
    nch_e = np.array(st["nch_tile"], np.int64)[t_e]
    if (j_e >= nch_e).any():
        raise PackError("chunk budget overflow")

    gpos_tab = np.zeros((T, B, 2), np.int64)
    ev_tab = np.zeros((T, B, 2), np.int64)
    for (t, b, j), v in gpos_of.items():
        gpos_tab[t, b, j] = v
    for (t, b, j), v in ev_of.items():
        ev_tab[t, b, j] = v
    gpos_e = gpos_tab[t_e, b_e, j_e]
    ev_e = ev_tab[t_e, b_e, j_e]

    # gather indices (global gather position i -> src row in bank), pad = 0
    idx_flat = np.zeros(n_pos * P, np.int64)
    idx_flat[gpos_e * P + p_e] = ss
    # int16 layout [128, n_pos*8]: value i at [i%16, i//16], replicated x8
    idx16 = np.zeros((P, n_pos * 8), np.int16)
    block = idx_flat.reshape(-1, 16).T.astype(np.int16)  # [16, n_pos*8]
    for k in range(8):
        idx16[k * 16 : (k + 1) * 16] = block

    # ld: local dst slot per chunk slot, -1 for dummies
    ld = np.full((P, n_ev), -1.0, np.float32)
    ld[p_e, ev_e] = slot_of[ds].astype(np.float32)

    # per-node scales
    ctot = cnt.sum(1).astype(np.float32)
    r8 = 1.0 / (8.0 * np.maximum(ctot, 1.0))

    # r8 replicated row per (tile, slot); holes -> 1/8 (slot output discarded)
    r8row = np.full(T * P, 1.0 / 8.0, np.float32)
    r8row[tile_of * P + slot_of] = r8
    r8rep = np.broadcast_to(r8row[None, :], (P, T * P))

    # folded residual in pm layout [slot, tile*D]
    xres_n = x_dst + (ctot * r8)[:, None] * bbig[None, :] + bout[None, :]
    xres = np.zeros((P, T * D), np.float32)
    xres[slot_of[:, None], (tile_of * D)[:, None] + np.arange(D)[None, :]] = xres_n

    return dict(
        idx=idx16,
        ld=np.ascontiguousarray(ld.astype(BF16)),
        r8rep=np.ascontiguousarray(r8rep.astype(BF16)),
        xres=np.ascontiguousarray(xres.astype(BF16)),
        tile_of=tile_of,
        slot_of=slot_of,
    )


def _fold_weights(Wv, bv, Wm, bm, Wout, bout):
    Wbig = (np.float32(Wv) @ np.float32(Wm)) @ np.float32(Wout)
    bbig = (np.float32(bv) @ np.float32(Wm) + np.float32(bm)) @ np.float32(Wout)
    return np.ascontiguousarray(Wbig).astype(BF16), bbig, np.float32(bout)


# ------------------------------------------------------------ device build

def _build(cfg, sts):
    f32 = mybir.dt.float32
    bf = mybir.dt.bfloat16
    i16 = mybir.dt.int16

    nc = bacc.Bacc(
        "TRN2",
        target_bir_lowering=False,
        debug=_DEBUG_BUILD,
        num_devices=cfg["ncores"],
        num_swdge_queues=4,
    )

    bank_cfg = dict(
        u=("xg", cfg["n_game"]),   # side u gathers from game banks
        g=("xu", cfg["n_user"]),   # side g gathers from user banks
    )
    sides = []
    for name in ("u", "g"):
        st = sts[name]
        pre, n_src = bank_cfg[name]
        B = st["B"]
        bank_sz = n_src // B
        side = dict(name=name, st=st, bank_sz=bank_sz)
        side["banks"] = [
            nc.dram_tensor(f"{pre}{b}", [bank_sz, D], bf, kind="ExternalInput")
            for b in range(B)
        ]
        T = st["T"]
        side["idx"] = nc.dram_tensor(f"idx_{name}", [P, st["n_pos"] * 8], i16, kind="ExternalInput")
        side["ld"] = nc.dram_tensor(f"ld_{name}", [P, st["n_ev"]], bf, kind="ExternalInput")
        side["r8"] = nc.dram_tensor(f"r8_{name}", [P, T * P], bf, kind="ExternalInput")
        side["xres"] = nc.dram_tensor(f"xres_{name}", [P, T * D], bf, kind="ExternalInput")
        side["w"] = nc.dram_tensor(f"w_{name}", [D, D], bf, kind="ExternalInput")
        side["out"] = nc.dram_tensor(f"out_{name}", [P, T * D], bf, kind="ExternalOutput")
        sides.append(side)

    iota_hbm = nc.dram_tensor("iota", [P, P], bf, kind="ExternalInput")
    ident_hbm = nc.dram_tensor("ident", [P, P], bf, kind="ExternalInput")

    max_seg_ch = max(
        info[3] for side in sides for info in side["st"]["seg_info"]
    )

    with tile.TileContext(nc) as tc, ExitStack() as ctx:
        const = ctx.enter_context(tc.tile_pool(name="const", bufs=1))
        gpool = ctx.enter_context(tc.tile_pool(name="gpool", bufs=16))
        mp = ctx.enter_context(tc.tile_pool(name="mp", bufs=2))
        stbp = ctx.enter_context(tc.tile_pool(name="stbp", bufs=3))
        xrp = ctx.enter_context(tc.tile_pool(name="xrp", bufs=2))
        oop = ctx.enter_context(tc.tile_pool(name="oop", bufs=3))
        outp = ctx.enter_context(tc.tile_pool(name="outp", bufs=2))
        st_ps = ctx.enter_context(tc.tile_pool(name="st_ps", bufs=4, space="PSUM"))
        op_ps = ctx.enter_context(tc.tile_pool(name="op_ps", bufs=3, space="PSUM"))

        iota_res = const.tile([P, P], bf, tag="iota", name="iota_res")
        nc.sync.dma_start(iota_res[:], iota_hbm[:])
        ident_res = const.tile([P, P], bf, tag="ident", name="ident_res")
        nc.sync.dma_start(ident_res[:], ident_hbm[:])
        gq = [0]  # round-robin SWDGE queue assignment for gathers

        # gather indices first: the first dma_gather depends only on idx_res
        for side in sides:
            st, n = side["st"], side["name"]
            side["idx_res"] = const.tile([P, st["n_pos"] * 8], i16, tag=f"idx_{n}", name=f"idx_res_{n}")
            nc.sync.dma_start(side["idx_res"][:], side["idx"][:])
        for side in sides:
            st, n = side["st"], side["name"]
            side["ld_res"] = const.tile([P, st["n_ev"]], bf, tag=f"ld_{n}", name=f"ld_res_{n}")
            nc.sync.dma_start(side["ld_res"][:], side["ld"][:])
            side["r8_res"] = const.tile([P, st["T"] * P], bf, tag=f"r8_{n}", name=f"r8_res_{n}")
            nc.sync.dma_start(side["r8_res"][:], side["r8"][:])
            side["w0"] = const.tile([P, D], bf, tag=f"w0_{n}", name=f"w0_{n}")
            nc.sync.dma_start(side["w0"][:], side["w"][0:P, :])
            side["w1"] = const.tile([P, D], bf, tag=f"w1_{n}", name=f"w1_{n}")
            nc.sync.dma_start(side["w1"][:], side["w"][P:D, :])

        for side in sides:
            st, n = side["st"], side["name"]
            T, B = st["T"], st["B"]
            nch_tile, ev_of, gpos_of = st["nch_tile"], st["ev_of"], st["gpos_of"]
            idx_res, ld_res, r8_res = side["idx_res"], side["ld_res"], side["r8_res"]

            MAXCH = 8  # dma_gather HW ucode limit: 1024 idxs per instruction
            for (tlo, thi, gl, nch_seg, per_bank) in st["seg_info"]:
                ntile = thi - tlo
                # one pool tile per gather so writers are independent and the
                # 4 SWDGE queues' descriptor generation actually overlaps
                gtiles = {}
                bl_of = {}
                for (b, bl, nch_b) in per_bank:
                    bl_of[b] = bl
                    for si, off in enumerate(range(0, nch_b, MAXCH)):
                        nch = min(MAXCH, nch_b - off)
                        lo = bl + off
                        gt = gpool.tile([P, MAXCH * D], bf, tag="gbuf", name=f"gbuf_{n}")
                        gtiles[(b, si)] = gt
                        out3 = gt[:, 0 : nch * D].rearrange("p (c e) -> p c e", e=D)
                        nc.gpsimd.dma_gather(
                            out3,
                            side["banks"][b][:, :],
                            idx_res[:, lo * 8 : (lo + nch) * 8],
                            nch * P,
                            nch * P,
                            D,
                            queue_num=gq[0] % 4,
                        )
                        gq[0] += 1

                xr = xrp.tile([P, ntile * D], bf, tag="xr", name=f"xr_{n}")
                nc.sync.dma_start(xr[:], side["xres"][:, tlo * D : thi * D])
                og = outp.tile([P, ntile * D], bf, tag="og", name=f"og_{n}")

                # one-hot selection matrices for the whole segment in one DVE op
                ev0s = ev_of[(tlo, 0, 0)]
                ev1s = ev_of[(thi - 1, B - 1, nch_tile[thi - 1] - 1)] + 1
                nevs = ev1s - ev0s
                Mt = mp.tile([P, nevs * P], bf, tag="m", name=f"m_{n}")
                nc.vector.tensor_tensor(
                    out=Mt[:].rearrange("p (c j) -> p c j", j=P),
                    in0=ld_res[:, ev0s:ev1s].unsqueeze(2).to_broadcast([P, nevs, P]),
                    in1=iota_res[:].unsqueeze(1).to_broadcast([P, nevs, P]),
                    op=mybir.AluOpType.is_equal,
                )

                def do_tile(t, opre_ap):
                    """stage-1 scatter matmuls + r8 scale + stage-2 + residual"""
                    Ct = nch_tile[t] * B
                    ti_ = t - tlo
                    stp = st_ps.tile([P, D], f32, tag="st")
                    for h in range(2):  # feature halves: sequential PSUM groups
                        k = 0
                        for b in range(B):
                            for j in range(nch_tile[t]):
                                lp = gpos_of[(t, b, j)] - bl_of[b]
                                kM = ev_of[(t, b, j)] - ev0s
                                gt = gtiles[(b, lp // MAXCH)]
                                lc = lp % MAXCH
                                X = gt[:, lc * D + h * P : lc * D + (h + 1) * P]
                                nc.tensor.matmul(
                                    stp[:, h * P : (h + 1) * P], lhsT=X,
                                    rhs=Mt[:, kM * P : (kM + 1) * P],
                                    start=(k == 0), stop=(k == Ct - 1),
                                )
                                k += 1
                    stb = stbp.tile([P, D], bf, tag="stb", name=f"stb_{n}")
                    nc.vector.tensor_tensor(
                        out=stb[:].rearrange("p (h j) -> p h j", j=P),
                        in0=stp[:].rearrange("p (h j) -> p h j", j=P),
                        in1=r8_res[:, t * P : (t + 1) * P].unsqueeze(1).to_broadcast([P, 2, P]),
                        op=mybir.AluOpType.mult,
                    )
                    nc.tensor.matmul(opre_ap, lhsT=stb[:, 0:P], rhs=side["w0"][:], start=True, stop=False)
                    nc.tensor.matmul(opre_ap, lhsT=stb[:, P:D], rhs=side["w1"][:], start=False, stop=True)

                for t0 in range(tlo, thi, 2):
                    npair = min(2, thi - t0)
                    W = npair * D
                    ti = t0 - tlo
                    opre = op_ps.tile([P, W], f32, tag="opre")
                    for q in range(npair):
                        do_tile(t0 + q, opre[:, q * D : (q + 1) * D])
                    oo = oop.tile([P, W], f32, tag="oo", name=f"oo_{n}")
                    nc.vector.tensor_tensor(
                        out=oo[:], in0=opre[:], in1=xr[:, ti * D : ti * D + W],
                        op=mybir.AluOpType.add,
                    )
                    nc.scalar.activation(
                        og[:, ti * D : ti * D + W], oo[:],
                        mybir.ActivationFunctionType.Relu,
                    )

                nc.sync.dma_start(side["out"][:, tlo * D : thi * D], og[:])

    nc.compile()
    return nc


_NC_CACHE = {}


def _cfg_key(cfg):
    return (
        cfg["ncores"], cfg["n_user"], cfg["n_game"],
        tuple(sorted(cfg["u"].items())), tuple(sorted(cfg["g"].items())),
    )


def _get_nc(cfg, sts):
    key = _cfg_key(cfg)
    if key not in _NC_CACHE:
        _NC_CACHE[key] = _build(cfg, sts)
    return _NC_CACHE[key]


# ------------------------------------------------------------------- driver

def _prepare(inputs, cfg):
    ncores = cfg["ncores"]
    n_user, n_game = cfg["n_user"], cfg["n_game"]
    uslice, gslice = n_user // ncores, n_game // ncores

    Wb_u, bb_u, bo_u = _fold_weights(
        inputs["Wv_game"], inputs["bv_game"], inputs["Wm_rev"], inputs["bm_rev"],
        inputs["Wout_user"], inputs["bout_user"],
    )
    Wb_g, bb_g, bo_g = _fold_weights(
        inputs["Wv_user"], inputs["bv_user"], inputs["Wm_played"], inputs["bm_played"],
        inputs["Wout_game"], inputs["bout_game"],
    )

    x_user = np.ascontiguousarray(np.float32(inputs["x_user"]))
    x_game = np.ascontiguousarray(np.float32(inputs["x_game"]))
    xu_bf = x_user.astype(BF16)
    xg_bf = x_game.astype(BF16)

    ep_s = np.asarray(inputs["ei_played_src"]).astype(np.int64)
    ep_d = np.asarray(inputs["ei_played_dst"]).astype(np.int64)
    er_s = np.asarray(inputs["ei_rev_src"]).astype(np.int64)
    er_d = np.asarray(inputs["ei_rev_dst"]).astype(np.int64)

    while True:
        sts = _structures(cfg)
        try:
            in_maps = []
            packs = []
            for k in range(ncores):
                sel_u = (er_d >= k * uslice) & (er_d < (k + 1) * uslice)
                pu = _pack_side(
                    sts["u"], er_d[sel_u] - k * uslice, er_s[sel_u],
                    n_game // cfg["u"]["nbanks"], uslice,
                    x_user[k * uslice : (k + 1) * uslice],
                    np.float32(Wb_u), bb_u, bo_u,
                )
                sel_g = (ep_d >= k * gslice) & (ep_d < (k + 1) * gslice)
                pg = _pack_side(
                    sts["g"], ep_d[sel_g] - k * gslice, ep_s[sel_g],
                    n_user // cfg["g"]["nbanks"], gslice,
                    x_game[k * gslice : (k + 1) * gslice],
                    np.float32(Wb_g), bb_g, bo_g,
                )
                packs.append((pu, pg))
                im = dict(
                    iota=np.ascontiguousarray(
                        np.broadcast_to(np.arange(P, dtype=np.float32)[None, :], (P, P))
                    ).astype(BF16),
                    ident=np.eye(P, dtype=np.float32).astype(BF16),
                    idx_u=pu["idx"], ld_u=pu["ld"], r8_u=pu["r8rep"], xres_u=pu["xres"],
                    w_u=Wb_u,
                    idx_g=pg["idx"], ld_g=pg["ld"], r8_g=pg["r8rep"], xres_g=pg["xres"],
                    w_g=Wb_g,
                )
                ubank = n_game // cfg["u"]["nbanks"]
                for b in range(cfg["u"]["nbanks"]):
                    im[f"xg{b}"] = xg_bf[b * ubank : (b + 1) * ubank]
                gbank = n_user // cfg["g"]["nbanks"]
                for b in range(cfg["g"]["nbanks"]):
                    im[f"xu{b}"] = xu_bf[b * gbank : (b + 1) * gbank]
                in_maps.append(im)
            break
        except PackError:
            # escalate fat-tile budget (changes structure => recompile)
            cfg = dict(cfg, u=dict(cfg["u"]), g=dict(cfg["g"]))
            cfg["u"]["fat"] += 2
            cfg["u"]["T"] += 1
            cfg["g"]["fat"] += 2
            cfg["g"]["T"] += 1

    return cfg, sts, in_maps, packs


def _run(inputs, cfg=None, trace=False, **run_kwargs):
    cfg = cfg or CFG_FULL
    cfg, sts, in_maps, packs = _prepare(inputs, cfg)
    ncores = cfg["ncores"]
    uslice, gslice = cfg["n_user"] // ncores, cfg["n_game"] // ncores

    nc = _get_nc(cfg, sts)
    res = run_bass_kernel_spmd(nc, in_maps, list(range(ncores)), trace=trace, **run_kwargs)

    def unpack(a, pack, T, nrows):
        a3 = np.asarray(a, dtype=np.float32).reshape(P, T, D)
        return a3[pack["slot_of"], pack["tile_of"], :]

    out_user = np.concatenate(
        [unpack(res.results[k]["out_u"], packs[k][0], sts["u"]["T"], uslice) for k in range(ncores)],
        axis=0,
    )
    out_game = np.concatenate(
        [unpack(res.results[k]["out_g"], packs[k][1], sts["g"]["T"], gslice) for k in range(ncores)],
        axis=0,
    )
    full = np.concatenate([out_user, out_game], axis=0).astype(np.float32)
    return full, res


def kernel(**inputs) -> np.ndarray:
    out, _ = _run(inputs)
    return out


# revision 46
# speedup vs baseline: 1.0295x; 1.0252x over previous
"""HGTConv Trainium2 kernel (8 NeuronCores, dst-sharded, batched SWDGE gather).

Math: softmax over the H=8 head axis followed by attn.mean(axis=-1) is
identically 1/8, so the attention branch drops out:

    out_dst = relu( segsum_dst(x_src[src]) @ Wbig * r8 + xres' )
    Wbig  = Wv @ Wm @ Wout
    r8    = 1/(8*max(cnt,1))                       (per dst node)
    xres' = x_dst + (cnt*r8)*bbig + bout           (host-folded residual)
    bbig  = (bv @ Wm + bm) @ Wout

Sharding: each core owns 1/8 of user dst nodes and 1/8 of game dst nodes,
and receives exactly the edges pointing into them. No collectives.

Gather: source rows are fetched with batched `dma_gather` (SWDGE custom
instruction, ~1us fixed cost per call amortized over thousands of rows)
instead of one indirect DMA per 128 rows. int16 gather indices limit the
addressable range to 32768 rows, so each source table is split into banks
of 25000 rows and one gather per (segment, bank) is issued.

Dst-node-to-tile assignment is bin-packed on the host (LPT) so that each
(tile, bank) group holds at most 128 edges (one 128-slot chunk); a few
"fat" tiles with a 256-edge budget absorb heavy nodes. This gives a fixed
chunk structure shared by all 8 cores (single SPMD program) at ~90-95%
gather-slot utilization.

Scatter within a tile: one-hot matmul. For each chunk, a [128 slots, 128
dst] selection matrix M (DVE is_equal of per-slot local-dst vs an iota
row) turns the segment-sum into PE matmuls accumulating S^T in PSUM; the
r8 scale rides the PSUM->SBUF copy (free-dim multiply with a
host-replicated r8 row); then a fused (Wbig) matmul, residual add, relu.
"""

import math
import os
from contextlib import ExitStack

import numpy as np
import ml_dtypes

import concourse.bass as bass
import concourse.tile as tile
import concourse.mybir as mybir
from concourse import bacc
from concourse.bass_utils import run_bass_kernel_spmd

P = 128
D = 256
BF16 = ml_dtypes.bfloat16
MAXCH = 8  # dma_gather HW ucode limit: 1024 idxs per instruction

# full-size problem config; side u: dst=user src=game, side g: dst=game src=user
CFG_FULL = dict(
    ncores=8,
    n_user=100000,
    n_game=50000,
    u=dict(T=99, fat=4, nbanks=2, nsegs=6),
    g=dict(T=50, fat=4, nbanks=4, nsegs=6),
)


class PackError(Exception):
    pass


_DEBUG_BUILD = False  # set True for CoreSim runs (keeps debug info)


# ------------------------------------------------------------ structure

def _side_structure(scfg):
    """Core-independent chunk/event layout for one side."""
    T, fat, B, nsegs = scfg["T"], scfg["fat"], scfg["nbanks"], scfg["nsegs"]
    # fat tiles at the FRONT so the tail segments are light
    nch_tile = [2 if t < fat else 1 for t in range(T)]  # chunks per (t, b)

    # segments: contiguous tile ranges, tail segments small to shorten the
    # compute drain after the last gather completes
    w = [1.0] * (nsegs - 2) + [0.5, 0.25]
    cw = np.cumsum([0.0] + w) / sum(w)
    bounds = sorted({int(round(c * T)) for c in cw} | {0, T})
    segs = [(bounds[i], bounds[i + 1]) for i in range(len(bounds) - 1)]

    # event order (ld columns / matmul order): tile-major
    ev_of = {}
    n_ev = 0
    for t in range(T):
        for b in range(B):
            for j in range(nch_tile[t]):
                ev_of[(t, b, j)] = n_ev
                n_ev += 1

    # gather-position order: bank-major (each bank is one contiguous chunk
    # stream, split into <=MAXCH-chunk gathers that may cross segments)
    gpos_of = {}
    bank_base = []
    pos = 0
    for b in range(B):
        bank_base.append(pos)
        for t in range(T):
            for j in range(nch_tile[t]):
                gpos_of[(t, b, j)] = pos
                pos += 1
    n_pos = pos
    chunks_per_bank = n_pos // B

    # gather issue order: split-major, bank-minor (matches tile-major
    # consumption so the sliding gather-tile pool never deadlocks)
    nsplits = math.ceil(chunks_per_bank / MAXCH)
    gathers = []  # (b, split_idx, gpos_lo, nch)
    for s in range(nsplits):
        for b in range(B):
            lo = bank_base[b] + s * MAXCH
            nch = min(MAXCH, chunks_per_bank - s * MAXCH)
            if nch > 0:
                gathers.append((b, s, lo, nch))

    return dict(
        T=T, fat=fat, B=B, nch_tile=nch_tile, segs=segs,
        ev_of=ev_of, n_ev=n_ev, gpos_of=gpos_of, n_pos=n_pos,
        bank_base=bank_base, gathers=gathers,
    )


def _structures(cfg):
    return dict(u=_side_structure(cfg["u"]), g=_side_structure(cfg["g"]))


# ------------------------------------------------------------ host packing

def _lpt_binpack(cnt, T, nch_tile, cap_chunk=128):
    """Assign each dst node to a tile s.t. per-(tile,bank) load <= cap and
    <=128 nodes per tile. cnt: [n_nodes, B] int. Returns tile_of [n_nodes]."""
    n, B = cnt.shape
    if n > T * P:
        raise PackError(f"{n} nodes > {T * P} slots")
    caps = np.broadcast_to(
        (np.array(nch_tile, np.int64) * cap_chunk)[:, None], (T, B)
    ).copy()
    slots = np.full(T, P, np.int64)
    tile_of = np.full(n, -1, np.int64)
    tot = cnt.sum(1)
    order = np.argsort(-cnt.max(1), kind="stable")
    nz = order[tot[order] > 0]
    for node in nz:
        c = cnt[node]
        feas = (slots > 0) & (caps >= c).all(1)
        if not feas.any():
            raise PackError("no feasible tile (escalate fat budget)")
        score = caps.sum(1).astype(np.float64)
        score[~feas] = -1.0
        t = int(np.argmax(score))
        tile_of[node] = t
        caps[t] -= c
        slots[t] -= 1
    z = order[tot[order] == 0]
    zi = 0
    for t in range(T):
        k = int(slots[t])
        if k > 0 and zi < len(z):
            take = z[zi : zi + k]
            tile_of[take] = t
            slots[t] -= len(take)
            zi += len(take)
    if zi < len(z):
        raise PackError("not enough node slots")
    return tile_of


def _pack_side(st, dst_local, src, bank_sz, n_dst_slice, x_dst, Wbig, bbig, bout):
    """Host packing of one core-side. Returns dict of device arrays + unpack maps."""
    T, B, n_ev, n_pos = st["T"], st["B"], st["n_ev"], st["n_pos"]
    nch_tile, ev_of, gpos_of = st["nch_tile"], st["ev_of"], st["gpos_of"]

    bank = (src // bank_sz).astype(np.int64)
    sib = (src - bank * bank_sz).astype(np.int64)  # src row within bank

    cnt = np.zeros((n_dst_slice, B), np.int64)
    np.add.at(cnt, (dst_local, bank), 1)
    tile_of = _lpt_binpack(cnt, T, st["nch_tile"])

    # slot within tile: stable order by node id
    order = np.argsort(tile_of, kind="stable")
    slot_of = np.empty(n_dst_slice, np.int64)
    tile_sorted = tile_of[order]
    first = np.searchsorted(tile_sorted, np.arange(T))
    slot_of[order] = np.arange(n_dst_slice) - first[tile_sorted]
    assert slot_of.max() < P

    # group edges by (tile, bank), sorted by src row for DMA locality
    g = tile_of[dst_local] * B + bank
    eorder = np.lexsort((sib, g))
    gs = g[eorder]
    ds = dst_local[eorder]
    ss = sib[eorder]
    gfirst = np.searchsorted(gs, np.arange(T * B))
    gcount = np.diff(np.searchsorted(gs, np.arange(T * B + 1)))
    within = np.arange(len(gs)) - gfirst[gs]

    t_e = gs // B
    b_e = gs % B
    j_e = within >> 7
    p_e = within & 127
    nch_e = np.array(st["nch_tile"], np.int64)[t_e]
    if (j_e >= nch_e).any():
        raise PackError("chunk budget overflow")

    gpos_tab = np.zeros((T, B, 2), np.int64)
    ev_tab = np.zeros((T, B, 2), np.int64)
    for (t, b, j), v in gpos_of.items():
        gpos_tab[t, b, j] = v
    for (t, b, j), v in ev_of.items():
        ev_tab[t, b, j] = v
    gpos_e = gpos_tab[t_e, b_e, j_e]
    ev_e = ev_tab[t_e, b_e, j_e]

    # gather indices (global gather position i -> src row in bank), pad = 0
    idx_flat = np.zeros(n_pos * P, np.int64)
    idx_flat[gpos_e * P + p_e] = ss
    # int16 layout [128, n_pos*8]: value i at [i%16, i//16], replicated x8
    idx16 = np.zeros((P, n_pos * 8), np.int16)
    block = idx_flat.reshape(-1, 16).T.astype(np.int16)  # [16, n_pos*8]
    for k in range(8):
        idx16[k * 16 : (k + 1) * 16] = block

    # ld: local dst slot per chunk slot, -1 for dummies
    ld = np.full((P, n_ev), -1.0, np.float32)
    ld[p_e, ev_e] = slot_of[ds].astype(np.float32)

    # per-node scales
    ctot = cnt.sum(1).astype(np.float32)
    r8 = 1.0 / (8.0 * np.maximum(ctot, 1.0))

    # r8 replicated row per (tile, slot); holes -> 1/8 (slot output discarded)
    r8row = np.full(T * P, 1.0 / 8.0, np.float32)
    r8row[tile_of * P + slot_of] = r8
    r8rep = np.broadcast_to(r8row[None, :], (P, T * P))

    # folded residual in pm layout [slot, tile*D]
    xres_n = x_dst + (ctot * r8)[:, None] * bbig[None, :] + bout[None, :]
    xres = np.zeros((P, T * D), np.float32)
    xres[slot_of[:, None], (tile_of * D)[:, None] + np.arange(D)[None, :]] = xres_n

    return dict(
        idx=idx16,
        ld=np.ascontiguousarray(ld.astype(BF16)),
        r8rep=np.ascontiguousarray(r8rep.astype(BF16)),
        xres=np.ascontiguousarray(xres.astype(BF16)),
        tile_of=tile_of,
        slot_of=slot_of,
    )


def _fold_weights(Wv, bv, Wm, bm, Wout, bout):
    Wbig = (np.float32(Wv) @ np.float32(Wm)) @ np.float32(Wout)
    bbig = (np.float32(bv) @ np.float32(Wm) + np.float32(bm)) @ np.float32(Wout)
    return np.ascontiguousarray(Wbig).astype(BF16), bbig, np.float32(bout)


# ------------------------------------------------------------ device build

def _build(cfg, sts):
    f32 = mybir.dt.float32
    bf = mybir.dt.bfloat16
    i16 = mybir.dt.int16

    nc = bacc.Bacc(
        "TRN2",
        target_bir_lowering=False,
        debug=_DEBUG_BUILD,
        num_devices=cfg["ncores"],
        num_swdge_queues=4,
    )

    bank_cfg = dict(
        u=("xg", cfg["n_game"]),   # side u gathers from game banks
        g=("xu", cfg["n_user"]),   # side g gathers from user banks
    )
    sides = []
    for name in ("u", "g"):
        st = sts[name]
        pre, n_src = bank_cfg[name]
        B = st["B"]
        bank_sz = n_src // B
        side = dict(name=name, st=st, bank_sz=bank_sz)
        side["banks"] = [
            nc.dram_tensor(f"{pre}{b}", [bank_sz, D], bf, kind="ExternalInput")
            for b in range(B)
        ]
        T = st["T"]
        side["idx"] = nc.dram_tensor(f"idx_{name}", [P, st["n_pos"] * 8], i16, kind="ExternalInput")
        side["ld"] = nc.dram_tensor(f"ld_{name}", [P, st["n_ev"]], bf, kind="ExternalInput")
        side["r8"] = nc.dram_tensor(f"r8_{name}", [P, T * P], bf, kind="ExternalInput")
        side["xres"] = nc.dram_tensor(f"xres_{name}", [P, T * D], bf, kind="ExternalInput")
        side["w"] = nc.dram_tensor(f"w_{name}", [D, D], bf, kind="ExternalInput")
        side["out"] = nc.dram_tensor(f"out_{name}", [P, T * D], bf, kind="ExternalOutput")
        sides.append(side)

    iota_hbm = nc.dram_tensor("iota", [P, P], bf, kind="ExternalInput")
    ident_hbm = nc.dram_tensor("ident", [P, P], bf, kind="ExternalInput")

    with tile.TileContext(nc) as tc, ExitStack() as ctx:
        const = ctx.enter_context(tc.tile_pool(name="const", bufs=1))
        gpool = ctx.enter_context(tc.tile_pool(name="gpool", bufs=16))
        mp = ctx.enter_context(tc.tile_pool(name="mp", bufs=2))
        stbp = ctx.enter_context(tc.tile_pool(name="stbp", bufs=3))
        xrp = ctx.enter_context(tc.tile_pool(name="xrp", bufs=2))
        oop = ctx.enter_context(tc.tile_pool(name="oop", bufs=3))
        outp = ctx.enter_context(tc.tile_pool(name="outp", bufs=2))
        st_ps = ctx.enter_context(tc.tile_pool(name="st_ps", bufs=4, space="PSUM"))
        op_ps = ctx.enter_context(tc.tile_pool(name="op_ps", bufs=3, space="PSUM"))

        iota_res = const.tile([P, P], bf, tag="iota", name="iota_res")
        nc.sync.dma_start(iota_res[:], iota_hbm[:])
        ident_res = const.tile([P, P], bf, tag="ident", name="ident_res")
        nc.sync.dma_start(ident_res[:], ident_hbm[:])
        gq = [0]  # round-robin SWDGE queue assignment for gathers

        # gather indices first: the first dma_gather depends only on idx_res
        for side in sides:
            st, n = side["st"], side["name"]
            side["idx_res"] = const.tile([P, st["n_pos"] * 8], i16, tag=f"idx_{n}", name=f"idx_res_{n}")
            nc.sync.dma_start(side["idx_res"][:], side["idx"][:])
        for side in sides:
            st, n = side["st"], side["name"]
            side["ld_res"] = const.tile([P, st["n_ev"]], bf, tag=f"ld_{n}", name=f"ld_res_{n}")
            nc.sync.dma_start(side["ld_res"][:], side["ld"][:])
            side["r8_res"] = const.tile([P, st["T"] * P], bf, tag=f"r8_{n}", name=f"r8_res_{n}")
            nc.sync.dma_start(side["r8_res"][:], side["r8"][:])
            side["w0"] = const.tile([P, D], bf, tag=f"w0_{n}", name=f"w0_{n}")
            nc.sync.dma_start(side["w0"][:], side["w"][0:P, :])
            side["w1"] = const.tile([P, D], bf, tag=f"w1_{n}", name=f"w1_{n}")
            nc.sync.dma_start(side["w1"][:], side["w"][P:D, :])

        for side in sides:
            st, n = side["st"], side["name"]
            T, B = st["T"], st["B"]
            nch_tile, ev_of, gpos_of = st["nch_tile"], st["ev_of"], st["gpos_of"]
            idx_res, ld_res, r8_res = side["idx_res"], side["ld_res"], side["r8_res"]

            bank_base = st["bank_base"]

            # all of this side's gathers, issued up front in split-major
            # bank-minor order (matches tile-major consumption); the sliding
            # 16-deep gather-tile pool provides flow control
            gtiles = {}
            for (b, s, lo, nch) in st["gathers"]:
                gt = gpool.tile([P, MAXCH * D], bf, tag="gbuf", name=f"gbuf_{n}")
                gtiles[(b, s)] = gt
                out3 = gt[:, 0 : nch * D].rearrange("p (c e) -> p c e", e=D)
                nc.gpsimd.dma_gather(
                    out3,
                    side["banks"][b][:, :],
                    idx_res[:, lo * 8 : (lo + nch) * 8],
                    nch * P,
                    nch * P,
                    D,
                    queue_num=gq[0] % 4,
                )
                gq[0] += 1

            for (tlo, thi) in st["segs"]:
                ntile = thi - tlo
                xr = xrp.tile([P, ntile * D], bf, tag="xr", name=f"xr_{n}")
                nc.sync.dma_start(xr[:], side["xres"][:, tlo * D : thi * D])
                og = outp.tile([P, ntile * D], bf, tag="og", name=f"og_{n}")

                # one-hot selection matrices for the whole segment in one DVE op
                ev0s = ev_of[(tlo, 0, 0)]
                ev1s = ev_of[(thi - 1, B - 1, nch_tile[thi - 1] - 1)] + 1
                nevs = ev1s - ev0s
                Mt = mp.tile([P, nevs * P], bf, tag="m", name=f"m_{n}")
                nc.vector.tensor_tensor(
                    out=Mt[:].rearrange("p (c j) -> p c j", j=P),
                    in0=ld_res[:, ev0s:ev1s].unsqueeze(2).to_broadcast([P, nevs, P]),
                    in1=iota_res[:].unsqueeze(1).to_broadcast([P, nevs, P]),
                    op=mybir.AluOpType.is_equal,
                )

                def do_tile(t, opre_ap):
                    """stage-1 scatter matmuls + r8 scale + stage-2 + residual"""
                    Ct = nch_tile[t] * B
                    ti_ = t - tlo
                    stp = st_ps.tile([P, D], f32, tag="st")
                    for h in range(2):  # feature halves: sequential PSUM groups
                        k = 0
                        for b in range(B):
                            for j in range(nch_tile[t]):
                                lp = gpos_of[(t, b, j)] - bank_base[b]
                                kM = ev_of[(t, b, j)] - ev0s
                                gt = gtiles[(b, lp // MAXCH)]
                                lc = lp % MAXCH
                                X = gt[:, lc * D + h * P : lc * D + (h + 1) * P]
                                nc.tensor.matmul(
                                    stp[:, h * P : (h + 1) * P], lhsT=X,
                                    rhs=Mt[:, kM * P : (kM + 1) * P],
                                    start=(k == 0), stop=(k == Ct - 1),
                                )
                                k += 1
                    stb = stbp.tile([P, D], bf, tag="stb", name=f"stb_{n}")
                    nc.vector.tensor_tensor(
                        out=stb[:].rearrange("p (h j) -> p h j", j=P),
                        in0=stp[:].rearrange("p (h j) -> p h j", j=P),
                        in1=r8_res[:, t * P : (t + 1) * P].unsqueeze(1).to_broadcast([P, 2, P]),
                        op=mybir.AluOpType.mult,
                    )
                    nc.tensor.matmul(opre_ap, lhsT=stb[:, 0:P], rhs=side["w0"][:], start=True, stop=False)
                    nc.tensor.matmul(opre_ap, lhsT=stb[:, P:D], rhs=side["w1"][:], start=False, stop=True)

                for t0 in range(tlo, thi, 2):
                    npair = min(2, thi - t0)
                    W = npair * D
                    ti = t0 - tlo
                    opre = op_ps.tile([P, W], f32, tag="opre")
                    for q in range(npair):
                        do_tile(t0 + q, opre[:, q * D : (q + 1) * D])
                    oo = oop.tile([P, W], f32, tag="oo", name=f"oo_{n}")
                    nc.vector.tensor_tensor(
                        out=oo[:], in0=opre[:], in1=xr[:, ti * D : ti * D + W],
                        op=mybir.AluOpType.add,
                    )
                    nc.scalar.activation(
                        og[:, ti * D : ti * D + W], oo[:],
                        mybir.ActivationFunctionType.Relu,
                    )

                nc.sync.dma_start(side["out"][:, tlo * D : thi * D], og[:])

    nc.compile()
    return nc


_NC_CACHE = {}


def _cfg_key(cfg):
    return (
        cfg["ncores"], cfg["n_user"], cfg["n_game"],
        tuple(sorted(cfg["u"].items())), tuple(sorted(cfg["g"].items())),
    )


def _get_nc(cfg, sts):
    key = _cfg_key(cfg)
    if key not in _NC_CACHE:
        _NC_CACHE[key] = _build(cfg, sts)
    return _NC_CACHE[key]


# ------------------------------------------------------------------- driver

def _prepare(inputs, cfg):
    ncores = cfg["ncores"]
    n_user, n_game = cfg["n_user"], cfg["n_game"]
    uslice, gslice = n_user // ncores, n_game // ncores

    Wb_u, bb_u, bo_u = _fold_weights(
        inputs["Wv_game"], inputs["bv_game"], inputs["Wm_rev"], inputs["bm_rev"],
        inputs["Wout_user"], inputs["bout_user"],
    )
    Wb_g, bb_g, bo_g = _fold_weights(
        inputs["Wv_user"], inputs["bv_user"], inputs["Wm_played"], inputs["bm_played"],
        inputs["Wout_game"], inputs["bout_game"],
    )

    x_user = np.ascontiguousarray(np.float32(inputs["x_user"]))
    x_game = np.ascontiguousarray(np.float32(inputs["x_game"]))
    xu_bf = x_user.astype(BF16)
    xg_bf = x_game.astype(BF16)

    ep_s = np.asarray(inputs["ei_played_src"]).astype(np.int64)
    ep_d = np.asarray(inputs["ei_played_dst"]).astype(np.int64)
    er_s = np.asarray(inputs["ei_rev_src"]).astype(np.int64)
    er_d = np.asarray(inputs["ei_rev_dst"]).astype(np.int64)

    while True:
        sts = _structures(cfg)
        try:
            in_maps = []
            packs = []
            for k in range(ncores):
                sel_u = (er_d >= k * uslice) & (er_d < (k + 1) * uslice)
                pu = _pack_side(
                    sts["u"], er_d[sel_u] - k * uslice, er_s[sel_u],
                    n_game // cfg["u"]["nbanks"], uslice,
                    x_user[k * uslice : (k + 1) * uslice],
                    np.float32(Wb_u), bb_u, bo_u,
                )
                sel_g = (ep_d >= k * gslice) & (ep_d < (k + 1) * gslice)
                pg = _pack_side(
                    sts["g"], ep_d[sel_g] - k * gslice, ep_s[sel_g],
                    n_user // cfg["g"]["nbanks"], gslice,
                    x_game[k * gslice : (k + 1) * gslice],
                    np.float32(Wb_g), bb_g, bo_g,
                )
                packs.append((pu, pg))
                im = dict(
                    iota=np.ascontiguousarray(
                        np.broadcast_to(np.arange(P, dtype=np.float32)[None, :], (P, P))
                    ).astype(BF16),
                    ident=np.eye(P, dtype=np.float32).astype(BF16),
                    idx_u=pu["idx"], ld_u=pu["ld"], r8_u=pu["r8rep"], xres_u=pu["xres"],
                    w_u=Wb_u,
                    idx_g=pg["idx"], ld_g=pg["ld"], r8_g=pg["r8rep"], xres_g=pg["xres"],
                    w_g=Wb_g,
                )
                ubank = n_game // cfg["u"]["nbanks"]
                for b in range(cfg["u"]["nbanks"]):
                    im[f"xg{b}"] = xg_bf[b * ubank : (b + 1) * ubank]
                gbank = n_user // cfg["g"]["nbanks"]
                for b in range(cfg["g"]["nbanks"]):
                    im[f"xu{b}"] = xu_bf[b * gbank : (b + 1) * gbank]
                in_maps.append(im)
            break
        except PackError:
            # escalate fat-tile budget (changes structure => recompile)
            cfg = dict(cfg, u=dict(cfg["u"]), g=dict(cfg["g"]))
            cfg["u"]["fat"] += 2
            cfg["u"]["T"] += 1
            cfg["g"]["fat"] += 2
            cfg["g"]["T"] += 1

    return cfg, sts, in_maps, packs


def _run(inputs, cfg=None, trace=False, **run_kwargs):
    cfg = cfg or CFG_FULL
    cfg, sts, in_maps, packs = _prepare(inputs, cfg)
    ncores = cfg["ncores"]
    uslice, gslice = cfg["n_user"] // ncores, cfg["n_game"] // ncores

    nc = _get_nc(cfg, sts)
    res = run_bass_kernel_spmd(nc, in_maps, list(range(ncores)), trace=trace, **run_kwargs)

    def unpack(a, pack, T, nrows):
        a3 = np.asarray(a, dtype=np.float32).reshape(P, T, D)
        return a3[pack["slot_of"], pack["tile_of"], :]

    out_user = np.concatenate(
        [unpack(res.results[k]["out_u"], packs[k][0], sts["u"]["T"], uslice) for k in range(ncores)],
        axis=0,
    )
    out_game = np.concatenate(
        [unpack(res.results[k]["out_g"], packs[k][1], sts["g"]["T"], gslice) for k in range(ncores)],
        axis=0,
    )
    full = np.concatenate([out_user, out_game], axis=0).astype(np.float32)
    return full, res


def kernel(**inputs) -> np.ndarray:
    out, _ = _run(inputs)
    return out


# revision 47
# speedup vs baseline: 1.0392x; 1.0094x over previous
"""HGTConv Trainium2 kernel (8 NeuronCores, dst-sharded, batched SWDGE gather).

Math: softmax over the H=8 head axis followed by attn.mean(axis=-1) is
identically 1/8, so the attention branch drops out:

    out_dst = relu( segsum_dst(x_src[src]) @ Wbig * r8 + xres' )
    Wbig  = Wv @ Wm @ Wout
    r8    = 1/(8*max(cnt,1))                       (per dst node)
    xres' = x_dst + (cnt*r8)*bbig + bout           (host-folded residual)
    bbig  = (bv @ Wm + bm) @ Wout

Sharding: each core owns 1/8 of user dst nodes and 1/8 of game dst nodes,
and receives exactly the edges pointing into them. No collectives.

Gather: source rows are fetched with batched `dma_gather` (SWDGE custom
instruction, ~1us fixed cost per call amortized over thousands of rows)
instead of one indirect DMA per 128 rows. int16 gather indices limit the
addressable range to 32768 rows, so each source table is split into banks
of 25000 rows and one gather per (segment, bank) is issued.

Dst-node-to-tile assignment is bin-packed on the host (LPT) so that each
(tile, bank) group holds at most 128 edges (one 128-slot chunk); a few
"fat" tiles with a 256-edge budget absorb heavy nodes. This gives a fixed
chunk structure shared by all 8 cores (single SPMD program) at ~90-95%
gather-slot utilization.

Scatter within a tile: one-hot matmul. For each chunk, a [128 slots, 128
dst] selection matrix M (DVE is_equal of per-slot local-dst vs an iota
row) turns the segment-sum into PE matmuls accumulating S^T in PSUM; the
r8 scale rides the PSUM->SBUF copy (free-dim multiply with a
host-replicated r8 row); then a fused (Wbig) matmul, residual add, relu.
"""

import math
import os
from contextlib import ExitStack

import numpy as np
import ml_dtypes

import concourse.bass as bass
import concourse.tile as tile
import concourse.mybir as mybir
from concourse import bacc
from concourse.bass_utils import run_bass_kernel_spmd

P = 128
D = 256
BF16 = ml_dtypes.bfloat16
MAXCH = 8  # dma_gather HW ucode limit: 1024 idxs per instruction

# full-size problem config; side u: dst=user src=game, side g: dst=game src=user
CFG_FULL = dict(
    ncores=8,
    n_user=100000,
    n_game=50000,
    u=dict(T=99, fat=4, nbanks=2, nsegs=6),
    g=dict(T=50, fat=4, nbanks=4, nsegs=6),
)


class PackError(Exception):
    pass


_DEBUG_BUILD = False  # set True for CoreSim runs (keeps debug info)


# ------------------------------------------------------------ structure

def _side_structure(scfg):
    """Core-independent chunk/event layout for one side."""
    T, fat, B, nsegs = scfg["T"], scfg["fat"], scfg["nbanks"], scfg["nsegs"]
    # fat tiles at the FRONT so the tail segments are light
    nch_tile = [2 if t < fat else 1 for t in range(T)]  # chunks per (t, b)

    # segments: contiguous tile ranges, tail segments small to shorten the
    # compute drain after the last gather completes
    w = [1.0] * (nsegs - 2) + [0.5, 0.25]
    cw = np.cumsum([0.0] + w) / sum(w)
    bounds = sorted({int(round(c * T)) for c in cw} | {0, T})
    segs = [(bounds[i], bounds[i + 1]) for i in range(len(bounds) - 1)]

    # event order (ld columns / matmul order): tile-major
    ev_of = {}
    n_ev = 0
    for t in range(T):
        for b in range(B):
            for j in range(nch_tile[t]):
                ev_of[(t, b, j)] = n_ev
                n_ev += 1

    # gather-position order: bank-major (each bank is one contiguous chunk
    # stream, split into <=MAXCH-chunk gathers that may cross segments)
    gpos_of = {}
    bank_base = []
    pos = 0
    for b in range(B):
        bank_base.append(pos)
        for t in range(T):
            for j in range(nch_tile[t]):
                gpos_of[(t, b, j)] = pos
                pos += 1
    n_pos = pos
    chunks_per_bank = n_pos // B

    # gather issue order: split-major, bank-minor (matches tile-major
    # consumption so the sliding gather-tile pool never deadlocks)
    nsplits = math.ceil(chunks_per_bank / MAXCH)
    gathers = []  # (b, split_idx, gpos_lo, nch)
    for s in range(nsplits):
        for b in range(B):
            lo = bank_base[b] + s * MAXCH
            nch = min(MAXCH, chunks_per_bank - s * MAXCH)
            if nch > 0:
                gathers.append((b, s, lo, nch))

    return dict(
        T=T, fat=fat, B=B, nch_tile=nch_tile, segs=segs,
        ev_of=ev_of, n_ev=n_ev, gpos_of=gpos_of, n_pos=n_pos,
        bank_base=bank_base, gathers=gathers,
    )


def _structures(cfg):
    return dict(u=_side_structure(cfg["u"]), g=_side_structure(cfg["g"]))


# ------------------------------------------------------------ host packing

def _lpt_binpack(cnt, T, nch_tile, cap_chunk=128):
    """Assign each dst node to a tile s.t. per-(tile,bank) load <= cap and
    <=128 nodes per tile. cnt: [n_nodes, B] int. Returns tile_of [n_nodes]."""
    n, B = cnt.shape
    if n > T * P:
        raise PackError(f"{n} nodes > {T * P} slots")
    caps = np.broadcast_to(
        (np.array(nch_tile, np.int64) * cap_chunk)[:, None], (T, B)
    ).copy()
    slots = np.full(T, P, np.int64)
    tile_of = np.full(n, -1, np.int64)
    tot = cnt.sum(1)
    order = np.argsort(-cnt.max(1), kind="stable")
    nz = order[tot[order] > 0]
    for node in nz:
        c = cnt[node]
        feas = (slots > 0) & (caps >= c).all(1)
        if not feas.any():
            raise PackError("no feasible tile (escalate fat budget)")
        score = caps.sum(1).astype(np.float64)
        score[~feas] = -1.0
        t = int(np.argmax(score))
        tile_of[node] = t
        caps[t] -= c
        slots[t] -= 1
    z = order[tot[order] == 0]
    zi = 0
    for t in range(T):
        k = int(slots[t])
        if k > 0 and zi < len(z):
            take = z[zi : zi + k]
            tile_of[take] = t
            slots[t] -= len(take)
            zi += len(take)
    if zi < len(z):
        raise PackError("not enough node slots")
    return tile_of


def _pack_side(st, dst_local, src, bank_sz, n_dst_slice, x_dst, Wbig, bbig, bout):
    """Host packing of one core-side. Returns dict of device arrays + unpack maps."""
    T, B, n_ev, n_pos = st["T"], st["B"], st["n_ev"], st["n_pos"]
    nch_tile, ev_of, gpos_of = st["nch_tile"], st["ev_of"], st["gpos_of"]

    bank = (src // bank_sz).astype(np.int64)
    sib = (src - bank * bank_sz).astype(np.int64)  # src row within bank

    cnt = np.zeros((n_dst_slice, B), np.int64)
    np.add.at(cnt, (dst_local, bank), 1)
    tile_of = _lpt_binpack(cnt, T, st["nch_tile"])

    # slot within tile: stable order by node id
    order = np.argsort(tile_of, kind="stable")
    slot_of = np.empty(n_dst_slice, np.int64)
    tile_sorted = tile_of[order]
    first = np.searchsorted(tile_sorted, np.arange(T))
    slot_of[order] = np.arange(n_dst_slice) - first[tile_sorted]
    assert slot_of.max() < P

    # group edges by (tile, bank), sorted by src row for DMA locality
    g = tile_of[dst_local] * B + bank
    eorder = np.lexsort((sib, g))
    gs = g[eorder]
    ds = dst_local[eorder]
    ss = sib[eorder]
    gfirst = np.searchsorted(gs, np.arange(T * B))
    gcount = np.diff(np.searchsorted(gs, np.arange(T * B + 1)))
    within = np.arange(len(gs)) - gfirst[gs]

    t_e = gs // B
    b_e = gs % B
    j_e = within >> 7
    p_e = within & 127
    nch_e = np.array(st["nch_tile"], np.int64)[t_e]
    if (j_e >= nch_e).any():
        raise PackError("chunk budget overflow")

    gpos_tab = np.zeros((T, B, 2), np.int64)
    ev_tab = np.zeros((T, B, 2), np.int64)
    for (t, b, j), v in gpos_of.items():
        gpos_tab[t, b, j] = v
    for (t, b, j), v in ev_of.items():
        ev_tab[t, b, j] = v
    gpos_e = gpos_tab[t_e, b_e, j_e]
    ev_e = ev_tab[t_e, b_e, j_e]

    # gather indices (global gather position i -> src row in bank), pad = 0
    idx_flat = np.zeros(n_pos * P, np.int64)
    idx_flat[gpos_e * P + p_e] = ss
    # int16 layout [128, n_pos*8]: value i at [i%16, i//16], replicated x8
    idx16 = np.zeros((P, n_pos * 8), np.int16)
    block = idx_flat.reshape(-1, 16).T.astype(np.int16)  # [16, n_pos*8]
    for k in range(8):
        idx16[k * 16 : (k + 1) * 16] = block

    # ld: local dst slot per chunk slot, -1 for dummies
    ld = np.full((P, n_ev), -1.0, np.float32)
    ld[p_e, ev_e] = slot_of[ds].astype(np.float32)

    # per-node scales
    ctot = cnt.sum(1).astype(np.float32)
    r8 = 1.0 / (8.0 * np.maximum(ctot, 1.0))

    # r8 replicated row per (tile, slot); holes -> 1/8 (slot output discarded)
    r8row = np.full(T * P, 1.0 / 8.0, np.float32)
    r8row[tile_of * P + slot_of] = r8
    r8rep = np.broadcast_to(r8row[None, :], (P, T * P))

    # folded residual in pm layout [slot, tile*D]
    xres_n = x_dst + (ctot * r8)[:, None] * bbig[None, :] + bout[None, :]
    xres = np.zeros((P, T * D), np.float32)
    xres[slot_of[:, None], (tile_of * D)[:, None] + np.arange(D)[None, :]] = xres_n

    return dict(
        idx=idx16,
        ld=np.ascontiguousarray(ld.astype(np.int8)),
        r8rep=np.ascontiguousarray(r8rep.astype(BF16)),
        xres=np.ascontiguousarray(xres.astype(BF16)),
        tile_of=tile_of,
        slot_of=slot_of,
    )


def _fold_weights(Wv, bv, Wm, bm, Wout, bout):
    Wbig = (np.float32(Wv) @ np.float32(Wm)) @ np.float32(Wout)
    bbig = (np.float32(bv) @ np.float32(Wm) + np.float32(bm)) @ np.float32(Wout)
    return np.ascontiguousarray(Wbig).astype(BF16), bbig, np.float32(bout)


# ------------------------------------------------------------ device build

def _build(cfg, sts):
    f32 = mybir.dt.float32
    bf = mybir.dt.bfloat16
    i16 = mybir.dt.int16
    i8 = mybir.dt.int8

    nc = bacc.Bacc(
        "TRN2",
        target_bir_lowering=False,
        debug=_DEBUG_BUILD,
        num_devices=cfg["ncores"],
        num_swdge_queues=4,
    )

    bank_cfg = dict(
        u=("xg", cfg["n_game"]),   # side u gathers from game banks
        g=("xu", cfg["n_user"]),   # side g gathers from user banks
    )
    sides = []
    for name in ("u", "g"):
        st = sts[name]
        pre, n_src = bank_cfg[name]
        B = st["B"]
        bank_sz = n_src // B
        side = dict(name=name, st=st, bank_sz=bank_sz)
        side["banks"] = [
            nc.dram_tensor(f"{pre}{b}", [bank_sz, D], bf, kind="ExternalInput")
            for b in range(B)
        ]
        T = st["T"]
        side["idx"] = nc.dram_tensor(f"idx_{name}", [P, st["n_pos"] * 8], i16, kind="ExternalInput")
        side["ld"] = nc.dram_tensor(f"ld_{name}", [P, st["n_ev"]], i8, kind="ExternalInput")
        side["r8"] = nc.dram_tensor(f"r8_{name}", [P, T * P], bf, kind="ExternalInput")
        side["xres"] = nc.dram_tensor(f"xres_{name}", [P, T * D], bf, kind="ExternalInput")
        side["w"] = nc.dram_tensor(f"w_{name}", [D, D], bf, kind="ExternalInput")
        side["out"] = nc.dram_tensor(f"out_{name}", [P, T * D], bf, kind="ExternalOutput")
        sides.append(side)

    iota_hbm = nc.dram_tensor("iota", [P, P], i8, kind="ExternalInput")
    ident_hbm = nc.dram_tensor("ident", [P, P], bf, kind="ExternalInput")

    with tile.TileContext(nc) as tc, ExitStack() as ctx:
        const = ctx.enter_context(tc.tile_pool(name="const", bufs=1))
        gpool = ctx.enter_context(tc.tile_pool(name="gpool", bufs=16))
        mp = ctx.enter_context(tc.tile_pool(name="mp", bufs=2))
        stbp = ctx.enter_context(tc.tile_pool(name="stbp", bufs=3))
        xrp = ctx.enter_context(tc.tile_pool(name="xrp", bufs=2))
        oop = ctx.enter_context(tc.tile_pool(name="oop", bufs=3))
        outp = ctx.enter_context(tc.tile_pool(name="outp", bufs=2))
        st_ps = ctx.enter_context(tc.tile_pool(name="st_ps", bufs=4, space="PSUM"))
        op_ps = ctx.enter_context(tc.tile_pool(name="op_ps", bufs=3, space="PSUM"))

        iota_res = const.tile([P, P], i8, tag="iota", name="iota_res")
        nc.sync.dma_start(iota_res[:], iota_hbm[:])
        ident_res = const.tile([P, P], bf, tag="ident", name="ident_res")
        nc.sync.dma_start(ident_res[:], ident_hbm[:])
        gq = [0]  # round-robin SWDGE queue assignment for gathers

        # gather indices first: the first dma_gather depends only on idx_res
        for side in sides:
            st, n = side["st"], side["name"]
            side["idx_res"] = const.tile([P, st["n_pos"] * 8], i16, tag=f"idx_{n}", name=f"idx_res_{n}")
            nc.sync.dma_start(side["idx_res"][:], side["idx"][:])
        for side in sides:
            st, n = side["st"], side["name"]
            side["ld_res"] = const.tile([P, st["n_ev"]], i8, tag=f"ld_{n}", name=f"ld_res_{n}")
            nc.sync.dma_start(side["ld_res"][:], side["ld"][:])
            side["r8_res"] = const.tile([P, st["T"] * P], bf, tag=f"r8_{n}", name=f"r8_res_{n}")
            nc.sync.dma_start(side["r8_res"][:], side["r8"][:])
            side["w0"] = const.tile([P, D], bf, tag=f"w0_{n}", name=f"w0_{n}")
            nc.sync.dma_start(side["w0"][:], side["w"][0:P, :])
            side["w1"] = const.tile([P, D], bf, tag=f"w1_{n}", name=f"w1_{n}")
            nc.sync.dma_start(side["w1"][:], side["w"][P:D, :])

        for side in sides:
            st, n = side["st"], side["name"]
            T, B = st["T"], st["B"]
            nch_tile, ev_of, gpos_of = st["nch_tile"], st["ev_of"], st["gpos_of"]
            idx_res, ld_res, r8_res = side["idx_res"], side["ld_res"], side["r8_res"]

            bank_base = st["bank_base"]

            # all of this side's gathers, issued up front in split-major
            # bank-minor order (matches tile-major consumption); the sliding
            # 16-deep gather-tile pool provides flow control
            gtiles = {}
            for (b, s, lo, nch) in st["gathers"]:
                gt = gpool.tile([P, MAXCH * D], bf, tag="gbuf", name=f"gbuf_{n}")
                gtiles[(b, s)] = gt
                out3 = gt[:, 0 : nch * D].rearrange("p (c e) -> p c e", e=D)
                nc.gpsimd.dma_gather(
                    out3,
                    side["banks"][b][:, :],
                    idx_res[:, lo * 8 : (lo + nch) * 8],
                    nch * P,
                    nch * P,
                    D,
                    queue_num=gq[0] % 4,
                )
                gq[0] += 1

            for (tlo, thi) in st["segs"]:
                ntile = thi - tlo
                xr = xrp.tile([P, ntile * D], bf, tag="xr", name=f"xr_{n}")
                nc.sync.dma_start(xr[:], side["xres"][:, tlo * D : thi * D])
                og = outp.tile([P, ntile * D], bf, tag="og", name=f"og_{n}")

                # one-hot selection matrices for the whole segment in one DVE op
                ev0s = ev_of[(tlo, 0, 0)]
                ev1s = ev_of[(thi - 1, B - 1, nch_tile[thi - 1] - 1)] + 1
                nevs = ev1s - ev0s
                Mt = mp.tile([P, nevs * P], bf, tag="m", name=f"m_{n}")
                nc.vector.tensor_tensor(
                    out=Mt[:].rearrange("p (c j) -> p c j", j=P),
                    in0=ld_res[:, ev0s:ev1s].unsqueeze(2).to_broadcast([P, nevs, P]),
                    in1=iota_res[:].unsqueeze(1).to_broadcast([P, nevs, P]),
                    op=mybir.AluOpType.is_equal,
                )

                def do_tile(t, opre_ap):
                    """stage-1 scatter matmuls + r8 scale + stage-2 + residual"""
                    Ct = nch_tile[t] * B
                    ti_ = t - tlo
                    stp = st_ps.tile([P, D], f32, tag="st")
                    for h in range(2):  # feature halves: sequential PSUM groups
                        k = 0
                        for b in range(B):
                            for j in range(nch_tile[t]):
                                lp = gpos_of[(t, b, j)] - bank_base[b]
                                kM = ev_of[(t, b, j)] - ev0s
                                gt = gtiles[(b, lp // MAXCH)]
                                lc = lp % MAXCH
                                X = gt[:, lc * D + h * P : lc * D + (h + 1) * P]
                                nc.tensor.matmul(
                                    stp[:, h * P : (h + 1) * P], lhsT=X,
                                    rhs=Mt[:, kM * P : (kM + 1) * P],
                                    start=(k == 0), stop=(k == Ct - 1),
                                )
                                k += 1
                    stb = stbp.tile([P, D], bf, tag="stb", name=f"stb_{n}")
                    nc.vector.tensor_tensor(
                        out=stb[:].rearrange("p (h j) -> p h j", j=P),
                        in0=stp[:].rearrange("p (h j) -> p h j", j=P),
                        in1=r8_res[:, t * P : (t + 1) * P].unsqueeze(1).to_broadcast([P, 2, P]),
                        op=mybir.AluOpType.mult,
                    )
                    nc.tensor.matmul(opre_ap, lhsT=stb[:, 0:P], rhs=side["w0"][:], start=True, stop=False)
                    nc.tensor.matmul(opre_ap, lhsT=stb[:, P:D], rhs=side["w1"][:], start=False, stop=True)

                for t0 in range(tlo, thi, 2):
                    npair = min(2, thi - t0)
                    W = npair * D
                    ti = t0 - tlo
                    opre = op_ps.tile([P, W], f32, tag="opre")
                    for q in range(npair):
                        do_tile(t0 + q, opre[:, q * D : (q + 1) * D])
                    oo = oop.tile([P, W], f32, tag="oo", name=f"oo_{n}")
                    nc.vector.tensor_tensor(
                        out=oo[:], in0=opre[:], in1=xr[:, ti * D : ti * D + W],
                        op=mybir.AluOpType.add,
                    )
                    nc.scalar.activation(
                        og[:, ti * D : ti * D + W], oo[:],
                        mybir.ActivationFunctionType.Relu,
                    )

                nc.sync.dma_start(side["out"][:, tlo * D : thi * D], og[:])

    nc.compile()
    return nc


_NC_CACHE = {}


def _cfg_key(cfg):
    return (
        cfg["ncores"], cfg["n_user"], cfg["n_game"],
        tuple(sorted(cfg["u"].items())), tuple(sorted(cfg["g"].items())),
    )


def _get_nc(cfg, sts):
    key = _cfg_key(cfg)
    if key not in _NC_CACHE:
        _NC_CACHE[key] = _build(cfg, sts)
    return _NC_CACHE[key]


# ------------------------------------------------------------------- driver

def _prepare(inputs, cfg):
    ncores = cfg["ncores"]
    n_user, n_game = cfg["n_user"], cfg["n_game"]
    uslice, gslice = n_user // ncores, n_game // ncores

    Wb_u, bb_u, bo_u = _fold_weights(
        inputs["Wv_game"], inputs["bv_game"], inputs["Wm_rev"], inputs["bm_rev"],
        inputs["Wout_user"], inputs["bout_user"],
    )
    Wb_g, bb_g, bo_g = _fold_weights(
        inputs["Wv_user"], inputs["bv_user"], inputs["Wm_played"], inputs["bm_played"],
        inputs["Wout_game"], inputs["bout_game"],
    )

    x_user = np.ascontiguousarray(np.float32(inputs["x_user"]))
    x_game = np.ascontiguousarray(np.float32(inputs["x_game"]))
    xu_bf = x_user.astype(BF16)
    xg_bf = x_game.astype(BF16)

    ep_s = np.asarray(inputs["ei_played_src"]).astype(np.int64)
    ep_d = np.asarray(inputs["ei_played_dst"]).astype(np.int64)
    er_s = np.asarray(inputs["ei_rev_src"]).astype(np.int64)
    er_d = np.asarray(inputs["ei_rev_dst"]).astype(np.int64)

    while True:
        sts = _structures(cfg)
        try:
            in_maps = []
            packs = []
            for k in range(ncores):
                sel_u = (er_d >= k * uslice) & (er_d < (k + 1) * uslice)
                pu = _pack_side(
                    sts["u"], er_d[sel_u] - k * uslice, er_s[sel_u],
                    n_game // cfg["u"]["nbanks"], uslice,
                    x_user[k * uslice : (k + 1) * uslice],
                    np.float32(Wb_u), bb_u, bo_u,
                )
                sel_g = (ep_d >= k * gslice) & (ep_d < (k + 1) * gslice)
                pg = _pack_side(
                    sts["g"], ep_d[sel_g] - k * gslice, ep_s[sel_g],
                    n_user // cfg["g"]["nbanks"], gslice,
                    x_game[k * gslice : (k + 1) * gslice],
                    np.float32(Wb_g), bb_g, bo_g,
                )
                packs.append((pu, pg))
                im = dict(
                    iota=np.ascontiguousarray(
                        np.broadcast_to(np.arange(P, dtype=np.int8)[None, :], (P, P))
                    ),
                    ident=np.eye(P, dtype=np.float32).astype(BF16),
                    idx_u=pu["idx"], ld_u=pu["ld"], r8_u=pu["r8rep"], xres_u=pu["xres"],
                    w_u=Wb_u,
                    idx_g=pg["idx"], ld_g=pg["ld"], r8_g=pg["r8rep"], xres_g=pg["xres"],
                    w_g=Wb_g,
                )
                ubank = n_game // cfg["u"]["nbanks"]
                for b in range(cfg["u"]["nbanks"]):
                    im[f"xg{b}"] = xg_bf[b * ubank : (b + 1) * ubank]
                gbank = n_user // cfg["g"]["nbanks"]
                for b in range(cfg["g"]["nbanks"]):
                    im[f"xu{b}"] = xu_bf[b * gbank : (b + 1) * gbank]
                in_maps.append(im)
            break
        except PackError:
            # escalate fat-tile budget (changes structure => recompile)
            cfg = dict(cfg, u=dict(cfg["u"]), g=dict(cfg["g"]))
            cfg["u"]["fat"] += 2
            cfg["u"]["T"] += 1
            cfg["g"]["fat"] += 2
            cfg["g"]["T"] += 1

    return cfg, sts, in_maps, packs


def _run(inputs, cfg=None, trace=False, **run_kwargs):
    cfg = cfg or CFG_FULL
    cfg, sts, in_maps, packs = _prepare(inputs, cfg)
    ncores = cfg["ncores"]
    uslice, gslice = cfg["n_user"] // ncores, cfg["n_game"] // ncores

    nc = _get_nc(cfg, sts)
    res = run_bass_kernel_spmd(nc, in_maps, list(range(ncores)), trace=trace, **run_kwargs)

    def unpack(a, pack, T, nrows):
        a3 = np.asarray(a, dtype=np.float32).reshape(P, T, D)
        return a3[pack["slot_of"], pack["tile_of"], :]

    out_user = np.concatenate(
        [unpack(res.results[k]["out_u"], packs[k][0], sts["u"]["T"], uslice) for k in range(ncores)],
        axis=0,
    )
    out_game = np.concatenate(
        [unpack(res.results[k]["out_g"], packs[k][1], sts["g"]["T"], gslice) for k in range(ncores)],
        axis=0,
    )
    full = np.concatenate([out_user, out_game], axis=0).astype(np.float32)
    return full, res


def kernel(**inputs) -> np.ndarray:
    out, _ = _run(inputs)
    return out
